# revision 39
# baseline (speedup 1.0000x reference)
"""Trainium2 Bass kernel for nn_ATT0: out[b,i,d] = tanh(x[b,i,d] * mean_j x[b,j,d]).

Full input [512, 128, 64] f32; batch dim sharded across 8 NeuronCores
(64 batches/core).  The device pipeline runs in fp16 (harness tolerance
2e-2; fp16 keeps absmax error ~1e-3): the host casts f32->f16 before
upload and back after gather, halving HBM traffic -- the dominant cost.

Per-core layout: partition p = b*16 + q (b in [0,8) batches-in-subchunk,
q in [0,16) row-groups); each partition holds a [II=8, D=64] block
(1 KiB fp16 runs, full DMA rate).  A subchunk is 8 batches; NSUB=8
subchunks per core.

Pipeline (per subchunk c):
  SP   : in-DMAs (grouped per CONFIG in_chunks), then per-out-group
         HWDGE dma_starts gated on that group's tanh
  PE   : 8 accumulating matmuls psum[c] += sel.T @ xt[:,c,k,:]
         (k = row within the partition block; one PSUM bank per chunk).
         The 0/1*(1/I) selector folds the 16 partition groups AND
         replicates each batch's mean to all its partitions;
         accumulation over k folds the 8 rows.  So PSUM holds
         mean_j x[b,j,d] directly, f32 -- no DVE reduce tree at all
         (vs the previous tree kernel this frees ~1.7us of DVE time).
  DVE  : sel build (shift/eq), copy psum f32 -> sums_sb f16 (pure
         cast), then ot = xt * bcast(sums_sb)  (fp16 2x mode)
  ACT  : tanh per tanh_group -- the serializing engine; groups are
         sized so ACT runs gapless once started
  Pool : sel iota only.  (SWDGE trigger_dma outs -- which would cut
         ~1.2us from the tail -- wedge this device in every variant
         tried, including one byte-identical to the old passing
         baseline's; see CONFIG["trigger_outs"].)

Raw Bass (no Tile), explicit semaphores.
"""

from contextlib import ExitStack

import numpy as np

import concourse.bass as bass
from concourse import library_config, mybir
from concourse.bass_utils import run_bass_kernel_spmd

B, I, D = 512, 128, 64
N_CORES = 8
BPC = B // N_CORES   # 64 batches per core
NB0 = 8              # batches per subchunk (4 or 8; set via CONFIG["nb0"])
S = 128 // NB0
II = I // S
NSUB = BPC // NB0


def _set_nb0(nb0):
    global NB0, S, II, NSUB
    NB0 = nb0
    S = 128 // NB0       # partition row-groups
    II = I // S          # rows per partition block
    NSUB = BPC // NB0    # subchunks per core

# ---- tunables --------------------------------------------------------------
CONFIG = dict(
    # subchunks per in-DMA (sums to NSUB)
    in_chunks=[1, 2, 2, 3],
    # psum->sbuf scaled-copy groups (consecutive chunks) + engine
    #   'd' DVE, 'a' ACT.  Chunks 1,2 copy singly: each copy starts as
    # soon as ITS chunk's PE fold retires, feeding mult (1,2) ~300ns
    # earlier than a pair-copy gated on both.
    copy_groups=[(0,), (1,), (2,), (3, 4), (5, 6), (7,)],
    copy_cls=["d", "d", "d", "d", "d", "d"],
    # multiply groups (consecutive chunks) + engine: 'd' DVE via sums_sb
    # (fp16 2x), 'D' DVE direct from psum (f32, no copy needed)
    mult_groups=[(0,), (1, 2), (3, 4), (5, 6), (7,)],
    mult_cls=["d", "d", "d", "d", "d"],
    # tanh instruction groups (consecutive chunks per ACT op)
    tanh_groups=[[0], [1, 2], [3, 4], [5, 6], [7]],
    # out-DMA groups; None -> tanh_groups
    out_groups=None,
    # number of TRAILING out groups fired via pre-prepared SWDGE triggers
    # (tail latency ~40ns vs ~1.3us HWDGE).  0 = all outs via SP HWDGE.
    # NOTE: every trigger_outs>=1 variant tried (including a config whose
    # trigger machinery is byte-identical to the previously-passing
    # baseline) wedges this device (NRT_EXEC_UNIT_UNRECOVERABLE) when
    # combined with the PE-fold/8-bank-psum pipeline, so triggers stay
    # OFF.
    trigger_outs=0,
    dma_scratch=16384,
    # issuing queue per non-triggered out group: 's' SP, 'a' ACT, 'd' DVE.
    # None -> all 's'.  ('a' for the last group skips the act_sem->SP hop
    # and SP queue serialization on the tail.)
    out_cls=None,
)

_cache = {}


def _chunk_in_map(in_chunks):
    m, w = {}, 0
    for k, v in enumerate(in_chunks):
        for _ in range(v):
            m[w] = k
            w += 1
    return m


def _default_orders(cfg):
    """Per-engine op lists. Each op is a tuple (kind, ...).

    Pool: ('iota',), ('shift',), ('eq',), ('idx',), ('idxmin',),
          ('lib',), ('prep', j), ('trig', j)
    DVE : ('copy', k), ('mult', gi)   (per copy_cls/mult_cls)
    ACT : ('copy', k), ('tanh', gi)
    """
    copy_groups, copy_cls = cfg["copy_groups"], cfg["copy_cls"]
    mult_groups, mult_cls = cfg["mult_groups"], cfg["mult_cls"]
    tanh_groups = cfg["tanh_groups"]
    out_groups = cfg["out_groups"] or cfg["tanh_groups"]
    ntrig = min(cfg.get("trigger_outs", 0), len(out_groups))

    pool = [("iota",)]
    if ntrig:
        pool += [op for j in range(ntrig)
                 for op in (("idx", j), ("idxmin", j))]
        pool += [("lib",)]
        pool += [("prep", j) for j in range(ntrig)]
        pool += [("trig", j) for j in range(ntrig)]

    # shift/eq must run on DVE: walrus rejects logical_shift_right /
    # is_equal TensorScalarPtr on the Pool engine.
    dve, act = [("shift",), ("eq",)], []

    def engine_list(e):
        return {"d": dve, "a": act}[e]

    # interleave copies/mults/tanhs in chunk order
    copied = set()
    multed = set()
    ci = mi = ti = 0
    while ci < len(copy_groups) or mi < len(mult_groups) or ti < len(tanh_groups):
        progress = False
        if ci < len(copy_groups):
            k = copy_groups[ci]
            # emit copy as soon as it's next (skip if all its chunks are
            # covered by 'D' mult groups only -- still harmless to emit)
            needed = any(
                mult_cls[g] != "D"
                for g, mg in enumerate(mult_groups)
                if any(c in k for c in mg)
            )
            if not needed:
                copied.update(k)
                ci += 1
                continue
            engine_list(copy_cls[ci]).append(("copy", ci))
            copied.update(k)
            ci += 1
            progress = True
        if mi < len(mult_groups):
            mg = mult_groups[mi]
            if mult_cls[mi] == "D" or all(c in copied for c in mg):
                dve.append(("mult", mi))
                multed.update(mg)
                mi += 1
                progress = True
        if ti < len(tanh_groups):
            tg = tanh_groups[ti]
            if all(c in multed for c in tg):
                act.append(("tanh", ti))
                ti += 1
                progress = True
        if not progress:
            raise AssertionError("order builder stuck")
    # non-SP out issues: append after the interleave (their act_sem waits
    # provide the ordering; queue position only needs producer-first)
    out_cls = cfg.get("out_cls") or ["s"] * len(out_groups)
    for gi in range(len(out_groups) - ntrig):
        if out_cls[gi] == "a":
            act.append(("out", gi))
        elif out_cls[gi] == "d":
            dve.append(("out", gi))
    return dve, pool, act


def _validate_orders(cfg, dve_ops, pool_ops, act_ops):
    """Same-engine dependencies rely on in-order execution plus explicit
    cross_waits; validate producers precede consumers on each queue."""
    copy_groups = cfg["copy_groups"]
    mult_groups, mult_cls = cfg["mult_groups"], cfg["mult_cls"]
    tanh_groups = cfg["tanh_groups"]
    out_groups = cfg["out_groups"] or cfg["tanh_groups"]
    all_ops = {}
    for eng, ops in (("d", dve_ops), ("p", pool_ops), ("a", act_ops)):
        for pos, op in enumerate(ops):
            assert op not in all_ops, f"duplicate op {op}"
            all_ops[op] = (eng, pos)

    def before(a, b):
        (ea, pa), (eb, pb) = all_ops[a], all_ops[b]
        if ea != eb:
            return True     # cross-engine pairs carry sem waits
        return pa < pb

    copy_of_chunk = {}
    for k, g in enumerate(copy_groups):
        for c in g:
            copy_of_chunk[c] = k
    for gi, g in enumerate(mult_groups):
        assert ("mult", gi) in all_ops and all_ops[("mult", gi)][0] == "d"
        if mult_cls[gi] == "D":
            continue
        for c in g:
            cp = ("copy", copy_of_chunk[c])
            assert cp in all_ops, f"missing copy for chunk {c}"
            assert before(cp, ("mult", gi)), f"copy/mult order chunk {c}"
    for gi in range(len(tanh_groups)):
        assert ("tanh", gi) in all_ops and all_ops[("tanh", gi)][0] == "a"
    # Pool ordering: iota/idx/idxmin before lib; preps after lib ascending;
    # trig j after prep j, trigs ascending (FIFO order).
    ntrig = min(cfg.get("trigger_outs", 0), len(out_groups))
    pp = {op: all_ops[op][1] for op in pool_ops}
    assert all_ops[("shift",)][0] == "d" and all_ops[("eq",)][0] == "d"
    assert before(("iota",), ("shift",)) and before(("shift",), ("eq",))
    if ntrig:
        lib_pos = pp[("lib",)]
        for op in pool_ops:
            if op[0] in ("iota", "shift", "eq", "idx", "idxmin"):
                assert pp[op] < lib_pos, f"{op} must precede library switch"
            elif op[0] in ("prep", "trig"):
                assert pp[op] > lib_pos, f"{op} must follow library switch"
        prep_pos = [pp[("prep", j)] for j in range(ntrig)]
        trig_pos = [pp[("trig", j)] for j in range(ntrig)]
        assert prep_pos == sorted(prep_pos) and trig_pos == sorted(trig_pos)
        for j in range(ntrig):
            assert pp[("idx", j)] < pp[("idxmin", j)] < prep_pos[j]
            assert prep_pos[j] < trig_pos[j]
    # tanh order on ACT must be monotone in group index (trigger waits
    # act_sem in out-group order)
    tpos = [all_ops[("tanh", gi)][1] for gi in range(len(tanh_groups))]
    assert tpos == sorted(tpos), "tanh groups must stay in order on ACT"


def _build():
    cfg = CONFIG
    _set_nb0(cfg.get("nb0", 8))
    f16 = mybir.dt.float16
    f32 = mybir.dt.float32
    in_chunks = cfg["in_chunks"]
    assert sum(in_chunks) == NSUB
    nchunk = NSUB
    out_groups = cfg["out_groups"] or cfg["tanh_groups"]
    tanh_groups = cfg["tanh_groups"]
    assert sorted(c for g in tanh_groups for c in g) == list(range(nchunk))
    assert sorted(c for g in out_groups for c in g) == list(range(nchunk))
    ntrig_g = min(cfg.get("trigger_outs", 0), len(out_groups))
    out_cls = cfg.get("out_cls") or ["s"] * len(out_groups)
    in_of_chunk = _chunk_in_map(in_chunks)
    in_starts = np.cumsum([0] + list(in_chunks))[:-1]
    copy_groups, copy_cls = cfg["copy_groups"], cfg["copy_cls"]
    mult_groups, mult_cls = cfg["mult_groups"], cfg["mult_cls"]
    assert sorted(c for g in mult_groups for c in g) == list(range(nchunk))
    need_copy = {c for gi, g in enumerate(mult_groups)
                 for c in g if mult_cls[gi] != "D"}
    covered = sorted(c for g in copy_groups for c in g)
    assert len(set(covered)) == len(covered)
    assert need_copy <= set(covered), (need_copy, covered)

    copy_of_chunk = {}
    for k, g in enumerate(copy_groups):
        for c in g:
            copy_of_chunk[c] = k
    mult_of_chunk = {}
    for gi, g in enumerate(mult_groups):
        for c in g:
            mult_of_chunk[c] = gi

    dve_ops, pool_ops, act_ops = (cfg.get("orders") or _default_orders(cfg))
    _validate_orders(cfg, dve_ops, pool_ops, act_ops)

    # Default SWDGE ring (16384/16 = 1024 descriptors) exactly equals the
    # 8*128 scatter descriptors prepared before any trigger fires; at the
    # boundary the ring wraps and the device wedges nondeterministically.
    nc = bass.Bass(dynamic_dma_scratch_size=cfg.get("dma_scratch", 16384))
    x = nc.dram_tensor("x", [BPC, I, D], f16, kind="ExternalInput")
    y = nc.dram_tensor("y", [BPC, I, D], f16, kind="ExternalOutput")

    xw = x[:].rearrange("(w b) (q i) d -> b q w i d", b=NB0, q=S)
    yw = y[:].rearrange("(w b) (q i) d -> b q w i d", b=NB0, q=S)
    # flat [(batch, q)-rows, 1 KiB runs] view of y for dma_scatter_add:
    # row (w*8+b)*16+q == w*128 + p, so a trigger group's rows are a pure
    # iota starting at w0*128.
    yw2 = y[:].rearrange("bb (q i) d -> (bb q) (i d)", q=S)

    with ExitStack() as ctx:
        ec = ctx.enter_context
        ctx.enter_context(nc.allow_low_precision(reason="fp16 pipeline"))
        sel_t = ec(nc.sbuf_tensor("sel_t", [128, 128], f16))
        sel_i = ec(nc.sbuf_tensor("sel_i", [128, 128], mybir.dt.int32))
        xt = ec(nc.sbuf_tensor("xt", [128, NSUB, II, D], f16))
        ot = ec(nc.sbuf_tensor("ot", [128, NSUB, II, D], f16))
        sums_sb = ec(nc.sbuf_tensor("ss", [128, NSUB, D], f16))
        # One PSUM bank (2 KiB = 512 f32) per chunk: accumulation groups are
        # tracked per bank, so chunk c's 8 accumulating matmuls must not
        # share a bank with a chunk whose mean is being read concurrently.
        # With psum_banks < NSUB, chunks wrap (bank c%n, offset 64*(c//n));
        # wrapped pairs are far apart in time, so no accumulation overlap.
        pbanks = cfg.get("psum_banks", NSUB)
        psum_t = ec(nc.psum_tensor("sm", [128, pbanks, 512], f32))

        def pslice(c0, n, mid=None):
            """AP for chunks c0..c0+n-1 (contiguous, non-wrapping)."""
            b0, o0 = c0 % pbanks, 64 * (c0 // pbanks)
            assert b0 + n <= pbanks, (c0, n, pbanks)
            if mid is None:
                return psum_t[:, b0:b0 + n, o0:o0 + D]
            return psum_t[:, b0:b0 + n, None, o0:o0 + D]
        trig_groups = out_groups[len(out_groups) - ntrig_g:] if ntrig_g \
            else []
        idxs = {j: ec(nc.sbuf_tensor(f"ix{j}", [128, len(g) * NB0],
                                     mybir.dt.int16))
                for j, g in enumerate(trig_groups)}

        in_sems = [ec(nc.semaphore(f"in_sem{k}"))
                   for k in range(len(in_chunks))]
        trig_sem = ec(nc.semaphore("trig_sem"))
        out_sem = ec(nc.semaphore("out_sem"))
        dve_sem = ec(nc.semaphore("dve_sem"))
        pe_sem = ec(nc.semaphore("pe_sem"))
        act_sem = ec(nc.semaphore("act_sem"))
        pool_sem = ec(nc.semaphore("pool_sem"))
        block = ec(nc.Block())

        # --- tick bookkeeping --------------------------------------------
        ticks = {}      # op -> (engine_char, tick)
        sems = {"d": dve_sem, "p": pool_sem, "a": act_sem}
        for eng, ops in (("d", dve_ops), ("p", pool_ops), ("a", act_ops)):
            t = 0
            for op in ops:
                if op[0] in ("lib", "trig", "out"):
                    continue
                t += 1
                ticks[op] = (eng, t)

        chunk_tanh_group = {}
        for gi, tg in enumerate(tanh_groups):
            for c in tg:
                chunk_tanh_group[c] = gi
        out_tick = [max(ticks[("tanh", chunk_tanh_group[c])][1] for c in g)
                    for g in out_groups]
        ntrig = ntrig_g
        ntrig0 = len(out_groups) - ntrig   # first triggered out-group index

        def in_waits(engine, chunks):
            for k in sorted({in_of_chunk[c] for c in chunks}):
                engine.wait_ge(in_sems[k], 16)

        def cross_wait(engine, my_eng, producer_op):
            """Wait on a producer. Engine pipelines are deep: even a
            dependent op on the SAME queue needs a semaphore wait on its
            producer (the CoreSim race detector enforces this)."""
            p_eng, t = ticks[producer_op]
            engine.wait_ge(sems[p_eng], t)

        def emit(engine, my_eng, op):
            kind = op[0]
            if kind == "iota":
                engine.iota(
                    sel_i[:].rearrange("p (a b) -> p a b", b=S),
                    pattern=[[-S, NB0], [0, S]],
                    base=0, channel_multiplier=1,
                ).then_inc(sems[my_eng])
            elif kind == "shift":
                cross_wait(engine, my_eng, ("iota",))
                engine.tensor_scalar(
                    out=sel_i[:], in0=sel_i[:],
                    scalar1=S.bit_length() - 1, scalar2=None,
                    op0=mybir.AluOpType.logical_shift_right,
                ).then_inc(sems[my_eng])
            elif kind == "eq":
                cross_wait(engine, my_eng, ("shift",))
                # sel = (p//S == m//S) * (1/I): the mean's 1/I rides the
                # selector, so psum holds the mean directly.
                engine.tensor_scalar(
                    out=sel_t[:], in0=sel_i[:], scalar1=0, scalar2=1.0 / I,
                    op0=mybir.AluOpType.is_equal,
                    op1=mybir.AluOpType.mult,
                ).then_inc(sems[my_eng])
            elif kind == "idx":
                # scatter rows for trigger group j: idxs[p, s] = w0*128 +
                # s*S + p (only partitions 0..S-1 become tokens; token
                # (p=q, s=(w-w0)*NB0+b) -> y row w*128 + b*S + q)
                j = op[1]
                g = trig_groups[j]
                engine.iota(
                    idxs[j][:], pattern=[[S, len(g) * NB0]],
                    base=g[0] * 128, channel_multiplier=1,
                ).then_inc(sems[my_eng])
            elif kind == "idxmin":
                # clamp unused partitions' values into range for the
                # executor's bounds assert
                j = op[1]
                cross_wait(engine, my_eng, ("idx", j))
                engine.tensor_scalar(
                    out=idxs[j][:], in0=idxs[j][:],
                    scalar1=BPC * S - 1, scalar2=None,
                    op0=mybir.AluOpType.min,
                ).then_inc(sems[my_eng])
            elif kind == "lib":
                # Gate the library switch (and the SWDGE preps behind it)
                # until the in-stream completes: desc-gen concurrent with
                # the HWDGE in-DMA stream wedges the device.
                in_waits(engine, [NSUB - 1])
                engine.load_library(library_config.mlp)
            elif kind == "prep":
                j = op[1]
                g = trig_groups[j]
                c0, n = g[0], len(g)
                cross_wait(engine, my_eng, ("idxmin", j))
                engine.dma_scatter_add(
                    yw2, ot[:, c0:c0 + n].rearrange("p w i d -> p w (i d)"),
                    idxs[j][:],
                    num_idxs=n * 128, num_idxs_reg=n * 128,
                    elem_size=II * D,
                    prepare_only=True, sem=trig_sem,
                ).then_inc(sems[my_eng])
            elif kind == "trig":
                j = op[1]
                engine.wait_ge(act_sem, out_tick[ntrig0 + j])
                cross_wait(engine, my_eng, ("prep", j))
                engine.trigger_dma(count=1)
            elif kind == "out":
                gi = op[1]
                g = out_groups[gi]
                engine.wait_ge(act_sem, out_tick[gi])
                engine.dma_start(
                    yw[:, :, g[0]:g[0] + len(g)],
                    ot[:, g[0]:g[0] + len(g)],
                ).then_inc(out_sem, 16)
            elif kind == "copy":
                k = op[1]
                g = copy_groups[k]
                engine.wait_ge(pe_sem, max(g) + 1)
                src = pslice(g[0], len(g))
                dst = sums_sb[:, g[0]:g[0] + len(g)]
                if my_eng == "a":
                    engine.mul(dst, src, 1.0).then_inc(act_sem)
                else:
                    engine.tensor_scalar(
                        out=dst, in0=src, scalar1=1.0, scalar2=None,
                        op0=mybir.AluOpType.mult,
                    ).then_inc(sems[my_eng])
            elif kind == "mult":
                gi = op[1]
                g = mult_groups[gi]
                c0, n = g[0], len(g)
                if mult_cls[gi] == "D":   # direct from psum (f32, no 2x)
                    engine.wait_ge(pe_sem, max(g) + 1)
                    src = pslice(c0, n, mid=True).to_broadcast(
                        (128, n, II, D))
                else:
                    for k in sorted({copy_of_chunk[c] for c in g}):
                        cross_wait(engine, my_eng, ("copy", k))
                    src = sums_sb[:, c0:c0 + n, None, :].to_broadcast(
                        (128, n, II, D))
                engine.tensor_mul(
                    ot[:, c0:c0 + n], xt[:, c0:c0 + n], src
                ).then_inc(sems[my_eng])
            else:  # tanh
                gi = op[1]
                g = tanh_groups[gi]
                waits = {}
                for k in sorted({mult_of_chunk[c] for c in g}):
                    eng, t = ticks[("mult", k)]
                    waits[eng] = max(waits.get(eng, 0), t)
                for eng, t in sorted(waits.items()):
                    if eng != my_eng:
                        engine.wait_ge(sems[eng], t)
                tgt = ot[:, g[0]:g[0] + len(g)]
                engine.activation(
                    out=tgt, in_=tgt,
                    func=mybir.ActivationFunctionType.Tanh, scale=1.0,
                ).then_inc(act_sem)

        # --- engine programs ----------------------------------------------
        @block.sync
        def _(sync):
            for k in range(len(in_chunks)):
                w0 = in_starts[k]
                sync.dma_start(
                    xt[:, w0:w0 + in_chunks[k]],
                    xw[:, :, w0:w0 + in_chunks[k]],
                ).then_inc(in_sems[k], 16)
            for gi in range(ntrig0):
                if out_cls[gi] != "s":
                    continue
                g = out_groups[gi]
                sync.wait_ge(act_sem, out_tick[gi])
                sync.dma_start(
                    yw[:, :, g[0]:g[0] + len(g)],
                    ot[:, g[0]:g[0] + len(g)],
                ).then_inc(out_sem, 16)
            if ntrig0:
                sync.wait_ge(out_sem, 16 * ntrig0)
            if ntrig:
                sync.wait_ge(trig_sem, 16 * ntrig)

        @block.vector
        def _(vector):
            for op in dve_ops:
                emit(vector, "d", op)

        @block.tensor
        def _(tensor):
            eq_eng, eq_tick = ticks[("eq",)]
            tensor.wait_ge(sems[eq_eng], eq_tick)
            for c in range(nchunk):
                in_waits(tensor, [c])
                for k in range(II):
                    mm = tensor.matmul(
                        pslice(c, 1), sel_t[:], xt[:, c, k, :],
                        start=(k == 0), stop=(k == II - 1),
                    )
                    if k == II - 1:
                        mm.then_inc(pe_sem)

        @block.gpsimd
        def _(gpsimd):
            for op in pool_ops:
                emit(gpsimd, "p", op)

        @block.scalar
        def _(scalar):
            for op in act_ops:
                emit(scalar, "a", op)

    # Populate .instr bytes for extended-inst InstISA subclasses
    # (scatter preps / trigger) -- raw Bass skips Bacc's codegen pass
    # and walrus fails with "ISA wrong length" without it.
    from concourse.library_overlay import lower_extended_insts
    lower_extended_insts(nc)
    return nc


def _prepare():
    """Build the Bass module once and cache a jitted shard_map executable."""
    import jax
    from jax.experimental.shard_map import shard_map
    from jax.sharding import Mesh, PartitionSpec
    from concourse import bass2jax, mybir as mb

    nc = _build()
    bass2jax.install_neuronx_cc_hook()
    assert nc.dbg_addr is None

    partition_name = (nc.partition_id_tensor.name
                      if nc.partition_id_tensor else None)
    in_names, out_names, out_avals = [], [], []
    for alloc in nc.m.functions[0].allocations:
        if not isinstance(alloc, mb.MemoryLocationSet):
            continue
        name = alloc.memorylocations[0].name
        if alloc.kind == "ExternalInput":
            if name != partition_name:
                in_names.append(name)
        elif alloc.kind == "ExternalOutput":
            shape = tuple(alloc.tensor_shape)
            out_avals.append(
                jax.core.ShapedArray(shape, mb.dt.np(alloc.dtype)))
            out_names.append(name)
    n_params = len(in_names)
    all_in_names = in_names + out_names
    if partition_name is not None:
        all_in_names = all_in_names + [partition_name]
    donate = tuple(range(n_params, n_params + len(out_names)))

    def _body(*args):
        operands = list(args)
        if partition_name is not None:
            operands.append(bass2jax.partition_id_tensor())
        return tuple(bass2jax._bass_exec_p.bind(
            *operands,
            out_avals=tuple(out_avals),
            in_names=tuple(all_in_names),
            out_names=tuple(out_names),
            lowering_input_output_aliases=(),
            sim_require_finite=True,
            sim_require_nnan=True,
            nc=nc,
        ))

    devices = jax.devices()[:N_CORES]
    assert len(devices) == N_CORES, devices
    mesh = Mesh(np.asarray(devices), ("core",))
    nio = n_params + len(out_names)
    sharded = jax.jit(
        shard_map(_body, mesh=mesh,
                  in_specs=(PartitionSpec("core"),) * nio,
                  out_specs=(PartitionSpec("core"),) * len(out_names),
                  check_rep=False),
        donate_argnums=donate, keep_unused=True,
    )
    _cache.update(nc=nc, sharded=sharded, in_names=in_names,
                  out_names=out_names, out_avals=out_avals)


def _run(emb: np.ndarray, trace: bool = False):
    emb16 = np.ascontiguousarray(emb.astype(np.float16))
    if trace:  # profiling path via bass_utils (no cached jit)
        nc = _cache.get("nc") or _build()
        in_maps = [
            {"x": np.ascontiguousarray(emb16[c * BPC:(c + 1) * BPC])}
            for c in range(N_CORES)
        ]
        res = run_bass_kernel_spmd(nc, in_maps, list(range(N_CORES)),
                                   trace=True)
        out = np.concatenate([r["y"] for r in res.results], axis=0)
        return out.astype(np.float32), res

    if "sharded" not in _cache:
        _prepare()
    assert _cache["in_names"] == ["x"] and _cache["out_names"] == ["y"]
    zeros = [np.zeros((N_CORES * a.shape[0], *a.shape[1:]), a.dtype)
             for a in _cache["out_avals"]]
    out_arrs = _cache["sharded"](emb16, *zeros)
    return np.asarray(out_arrs[0]).astype(np.float32), None


def kernel(embeddings: np.ndarray) -> np.ndarray:
    emb = np.asarray(embeddings, dtype=np.float32)
    try:
        out, _ = _run(emb, trace=False)
    except Exception:
        # Fallback: stock per-call path (same NEFF, re-traced each call).
        emb16 = np.ascontiguousarray(emb.astype(np.float16))
        nc = _cache.get("nc") or _build()
        in_maps = [
            {"x": np.ascontiguousarray(emb16[c * BPC:(c + 1) * BPC])}
            for c in range(N_CORES)
        ]
        res = run_bass_kernel_spmd(nc, in_maps, list(range(N_CORES)))
        out = np.concatenate(
            [r["y"] for r in res.results], axis=0).astype(np.float32)
    return out
